# revision 17
# baseline (speedup 1.0000x reference)
"""Causal multi-head attention on 8 Trainium2 NeuronCores.

Problem: x[2,2048,1024] @ W_Q/K/V[1024,1024] -> 16-head causal attention
(d_head=64) -> @ W_O[1024,1024].

Sharding: tensor-parallel over heads. Core i owns heads 2i, 2i+1 — i.e.
columns [128i:128i+128) of W_Q/W_K/W_V and rows [128i:128i+128) of W_O.
Each core computes its partial output [1024, 4096] (transposed layout);
the host sums the 8 partials and un-transposes (the "all-reduce").

Device kernel (per core, all matmuls in float32r = full-rate fp32):
  1. Projections from xT [1024, 4096] (host pre-transposes x):
     QT/KT [128, 4096] = W.T @ xT; V in natural [token, dim] layout via
     PE transpose, with a ones-column appended per head (65-wide blocks)
     so the PV matmul also produces the softmax denominator for free.
  2. Flash-style causal attention with scores in [k, q] orientation:
     scoresT = KT.T-slice @ QT-slice, exp on ScalarE (no max-subtraction:
     scores ~ N(0,1), exp is safe in fp32), causal mask applied
     multiplicatively on the 4 diagonal chunk variants only, PV matmul
     accumulates [65, 512] (64 dims + denominator row) in PSUM.
  3. Normalize by the denominator row (reciprocal + partition broadcast),
     then outT_partial = W_O-slice.T @ attnT.
"""

import numpy as np

import concourse.bass as bass
import concourse.tile as tile
from concourse import bacc, mybir
from concourse.bass_utils import run_bass_kernel_spmd
from concourse.masks import make_identity

F32 = mybir.dt.float32
F32R = mybir.dt.float32r

N_CORES = 8
P = 128
D = 1024          # d_model
B = 2             # batch
S = 2048          # seq len
T = B * S         # total tokens = 4096
TT = 512          # token tile (free dim of matmuls)
NT = T // TT      # 8 token tiles
KD = D // P       # 8 contraction chunks for projections
JB = S // TT      # 4 q-tiles per batch
CB = S // P       # 16 k-chunks per batch
NCH = T // P      # 32 k-chunks total
H_LOC = 2         # heads per core
DH = 64           # head dim


def _body(tc):
    nc = tc.nc
    xT = nc.dram_tensor("xT", [D, T], F32R, kind="ExternalInput").ap()
    wq = nc.dram_tensor("wq", [D, P], F32R, kind="ExternalInput").ap()
    wk = nc.dram_tensor("wk", [D, P], F32R, kind="ExternalInput").ap()
    wv = nc.dram_tensor("wv", [D, P], F32R, kind="ExternalInput").ap()
    wo = nc.dram_tensor("wo", [P, D], F32R, kind="ExternalInput").ap()
    outT = nc.dram_tensor("outT", [D, T], F32, kind="ExternalOutput").ap()

    import contextlib
    with contextlib.ExitStack() as ctx:
        const = ctx.enter_context(tc.tile_pool(name="const", bufs=1))
        wpool = ctx.enter_context(tc.tile_pool(name="wpool", bufs=1))
        xpool = ctx.enter_context(tc.tile_pool(name="xpool", bufs=2))
        persist = ctx.enter_context(tc.tile_pool(name="persist", bufs=1))
        probs_p = ctx.enter_context(tc.tile_pool(name="probs", bufs=6))
        stage = ctx.enter_context(tc.tile_pool(name="stage", bufs=3))
        psum = ctx.enter_context(tc.tile_pool(name="psum", bufs=2, space="PSUM"))
        psum_s = ctx.enter_context(tc.tile_pool(name="psum_s", bufs=2, space="PSUM"))
        psum_pv = ctx.enter_context(tc.tile_pool(name="psum_pv", bufs=4, space="PSUM"))

        # --- constants -----------------------------------------------------
        identity = const.tile([P, P], F32)
        make_identity(nc, identity)

        # mask_band[k, q] = 1.0 if q >= k else 0.0 (lower-left triangular 0s)
        mask_band = const.tile([P, P], F32)
        nc.any.memset(mask_band[:], 1.0)
        nc.gpsimd.affine_select(
            out=mask_band[:],
            in_=mask_band[:],
            compare_op=mybir.AluOpType.is_ge,
            fill=0.0,
            base=0,
            pattern=[[1, P]],
            channel_multiplier=-1,
        )

        # --- weights -------------------------------------------------------
        wq_sb = wpool.tile([P, KD, P], F32R)
        nc.sync.dma_start(wq_sb[:], wq.rearrange("(o p) m -> p o m", p=P))
        wk_sb = wpool.tile([P, KD, P], F32R)
        nc.sync.dma_start(wk_sb[:], wk.rearrange("(o p) m -> p o m", p=P))
        wv_sb = wpool.tile([P, KD, P], F32R)
        nc.sync.dma_start(wv_sb[:], wv.rearrange("(o p) m -> p o m", p=P))
        wo_sb = wpool.tile([P, D], F32R)
        nc.sync.dma_start(wo_sb[:], wo)

        # --- persistent activations ---------------------------------------
        qT = persist.tile([P, T], F32R)       # [2h x 64d, tokens]
        kT = persist.tile([P, T], F32R)
        vn = persist.tile([P, NCH, 130], F32R)  # [token, chunk, d0|1|d1|1]
        attnT = persist.tile([P, T], F32R)
        for col in (DH, 2 * DH + 1):
            nc.scalar.activation(vn[:, :, col], vn[:, :, col],
                                 mybir.ActivationFunctionType.Identity,
                                 bias=1.0, scale=0.0)

        xT_r = xT.rearrange("(o p) n -> p o n", p=P)
        outT_r = outT.rearrange("(o p) n -> p o n", p=P)

        # --- phase 1: projections -----------------------------------------
        for t in range(NT):
            xt = xpool.tile([P, KD, TT], F32R)
            for c in range(KD):
                nc.sync.dma_start(xt[:, c, :], xT_r[:, c, bass.ts(t, TT)])
            for wsb, dstT in ((wq_sb, qT), (wk_sb, kT)):
                ps = psum.tile([P, TT], F32, tag="mm")
                for c in range(KD):
                    nc.tensor.matmul(ps[:], wsb[:, c, :], xt[:, c, :],
                                     start=(c == 0), stop=(c == KD - 1))
                nc.vector.tensor_copy(dstT[:, bass.ts(t, TT)], ps[:])
            # V: project, then PE-transpose into natural [token, dim] layout
            ps = psum.tile([P, TT], F32, tag="mm")
            for c in range(KD):
                nc.tensor.matmul(ps[:], wv_sb[:, c, :], xt[:, c, :],
                                 start=(c == 0), stop=(c == KD - 1))
            vt = stage.tile([P, TT], F32, tag="vt")
            nc.vector.tensor_copy(vt[:], ps[:])
            for s_ in range(4):
                pt = psum_s.tile([P, P], F32, tag="sps")
                nc.tensor.transpose(pt[:], vt[:, bass.ts(s_, P)], identity)
                ch = t * 4 + s_
                nc.vector.tensor_copy(vn[:, ch, 0:DH], pt[:, 0:DH])
                nc.vector.tensor_copy(vn[:, ch, DH + 1:2 * DH + 1],
                                      pt[:, DH:2 * DH])

        # --- phase 2: causal attention ------------------------------------
        # Lag-1 software pipeline: the PV matmul for chunk cb-1 is emitted
        # after the scores matmul for chunk cb, so the PE never waits on
        # ScalarE's exp. Causal masking for the diagonal chunk r touches only
        # the 128-wide band: exp the live suffix, memset the dead prefix,
        # triangular-mask the band.
        for j in range(NT):
            b, jb = divmod(j, JB)
            ncb = 4 * (jb + 1)
            jsl = bass.ts(j, TT)
            pvs = [psum_pv.tile([DH + 1, TT], F32, tag="pv",
                                name=f"pv_{j}_{h_}")
                   for h_ in range(H_LOC)]

            def pv_step(cb, prs_pair, j=j, b=b, jb=jb, ncb=ncb, pvs=pvs):
                c = CB * b + cb
                r = cb - 4 * jb
                lo = P * r if r > 0 else 0   # dead-prefix columns skipped
                for h in range(H_LOC):
                    nc.tensor.matmul(pvs[h][:, lo:],
                                     vn[:, c, bass.ds((DH + 1) * h, DH + 1)],
                                     prs_pair[h][:, lo:],
                                     start=(cb == 0), stop=(cb == ncb - 1))

            pending = None
            for cb in range(ncb):
                c = CB * b + cb
                csl = bass.ts(c, P)
                r = cb - 4 * jb
                lo = P * r if r > 0 else 0
                prs_pair = []
                for h in range(H_LOC):
                    hp = slice(DH * h, DH * h + DH)
                    sps = psum_s.tile([P, TT], F32, tag="sps",
                                      name=f"sps_{j}_{cb}_{h}")
                    nc.tensor.matmul(sps[:, lo:], kT[hp, csl],
                                     qT[hp, jsl][:, lo:],
                                     start=True, stop=True)
                    pr = probs_p.tile([P, TT], F32R, tag="pr",
                                      name=f"pr_{j}_{cb}_{h}")
                    nc.scalar.activation(pr[:, lo:], sps[:, lo:],
                                         mybir.ActivationFunctionType.Exp,
                                         scale=0.125)
                    if r >= 0:
                        nc.vector.tensor_mul(pr[:, bass.ts(r, P)],
                                             pr[:, bass.ts(r, P)],
                                             mask_band[:])
                    prs_pair.append(pr)
                if pending is not None:
                    pv_step(cb - 1, pending)
                pending = prs_pair
            pv_step(ncb - 1, pending)

            for h in range(H_LOC):
                hp = slice(DH * h, DH * h + DH)
                # den -> SBUF (short DVE op), then reciprocal + partition
                # broadcast both on GpSimd, keeping the DVE FIFO free for
                # the phase-2 mask multiplies and ScalarE free for exp.
                rc = stage.tile([1, TT], F32, tag="rc", name=f"rc_{j}_{h}")
                nc.vector.reciprocal(rc[:], pvs[h][DH:DH + 1, :])
                rb = stage.tile([DH, TT], F32, tag="rb", name=f"rb_{j}_{h}")
                nc.gpsimd.partition_broadcast(rb[:], rc[:])
                nc.vector.tensor_mul(attnT[hp, jsl], pvs[h][0:DH, :], rb[:])

            # output projection for this q-tile, interleaved to keep the PE
            # dense (and the HAM clock warm) through the attention phase
            for f in range(KD):
                wps = psum.tile([P, TT], F32, tag="mm", name=f"wps_{j}_{f}")
                nc.tensor.matmul(wps[:], wo_sb[:, bass.ts(f, P)],
                                 attnT[:, jsl],
                                 start=True, stop=True)
                ob = stage.tile([P, TT], F32, tag="ob", name=f"ob_{j}_{f}")
                nc.vector.tensor_copy(ob[:], wps[:])
                nc.sync.dma_start(outT_r[:, f, bass.ts(j, TT)], ob[:])

_NC_CACHE = None


def _get_nc():
    global _NC_CACHE
    if _NC_CACHE is None:
        nc = bacc.Bacc("TRN2", target_bir_lowering=False, debug=False,
                       num_devices=N_CORES)
        with tile.TileContext(nc) as tc:
            _body(tc)
        nc.compile()
        _NC_CACHE = nc
    return _NC_CACHE


def _in_maps(x, W_Q, W_K, W_V, W_O):
    xT = np.ascontiguousarray(
        np.asarray(x, dtype=np.float32).reshape(T, D).T)
    W_Q = np.asarray(W_Q, dtype=np.float32)
    W_K = np.asarray(W_K, dtype=np.float32)
    W_V = np.asarray(W_V, dtype=np.float32)
    W_O = np.asarray(W_O, dtype=np.float32)
    maps = []
    for i in range(N_CORES):
        sl = slice(P * i, P * i + P)
        maps.append({
            "xT": xT,
            "wq": np.ascontiguousarray(W_Q[:, sl]),
            "wk": np.ascontiguousarray(W_K[:, sl]),
            "wv": np.ascontiguousarray(W_V[:, sl]),
            "wo": np.ascontiguousarray(W_O[sl, :]),
        })
    return maps


def _gather(results):
    acc = np.zeros([D, T], np.float64)
    for r in results:
        acc += r["outT"]
    return np.ascontiguousarray(
        acc.T.astype(np.float32)).reshape(B, S, D)


def kernel(x, W_Q, W_K, W_V, W_O):
    nc = _get_nc()
    res = run_bass_kernel_spmd(nc, _in_maps(x, W_Q, W_K, W_V, W_O),
                               core_ids=list(range(N_CORES)))
    return _gather(res.results)


def kernel_profiled(x, W_Q, W_K, W_V, W_O):
    """Like kernel() but with NTFF tracing; returns (output, exec_time_ns)."""
    nc = _get_nc()
    res = run_bass_kernel_spmd(nc, _in_maps(x, W_Q, W_K, W_V, W_O),
                               core_ids=list(range(N_CORES)), trace=True)
    return _gather(res.results), res.exec_time_ns
